# revision 8
# baseline (speedup 1.0000x reference)
"""Trainium2 Bass kernel for nn_AutoIntTPPSameInfluence — exp-sum formulation.

dF(x) (the scalar derivative of the 1->64->64->64->1 tanh MLP) is fit on host
as a sum of 8 decaying exponentials  dF(x) ~= sum_i c_i exp(-s_i x)  (ridge
LSQ over a geomspace rate grid).  On device every pairwise influence point is
then ONE table-exp evaluation: points are packed 16-per-segment-column and
replicated over the 8 partition groups of a [128, N] tile, a single ACT
instruction computes |c_i| exp(-s_i x + ln|c_i|) for all 8 rates via
per-partition scale/bias, and one bf16 matmul with a +-1 selector contracts
all 128 partitions — performing the 16-point segment sum AND the 8-term
weighted sum at 1 col/cycle.  Masked/padded points use x = 3e4, driving every
exponential to exactly 0.

The integral term sum_k F(T_END - t_k) - F0 only needs a bulk sum, so
F(x) - F0 is fit the same way (const + 8 exps) and rides through the same
pipeline as the leading FT columns; the constant is applied on host via the
valid count.  Host: scatter segment sums to events, log/mask/reduce in f64.

Schedule notes: a dummy ACT on a framework const AP forces the exp
ACT_TABLE_LOAD to run during the input-DMA completion window (~2.5 us fixed
DMA latency); F columns lead so their chain never sits in the tail; PSUM
accumulation groups of 4 tiles let output DMAs start mid-run.
"""

import numpy as np
from contextlib import ExitStack

import ml_dtypes

import concourse.bass as bass
import concourse.bacc as bacc
import concourse.tile as tile
import concourse.mybir as mybir
from concourse.bass_utils import run_bass_kernel_spmd

B, L, H = 16, 320, 64
T_END = 100.0
NC = 8
SEG = 16                    # points per segment column
K = 8                       # exponential rates (K * SEG == 128)
GS = 4                      # tiles per PSUM accumulation group
TS_MAX = 512                # PSUM bank width in f32
XPAD = np.float32(30000.0)  # pad x: exp(-s*XPAD) underflows to exactly 0
BF16 = mybir.dt.bfloat16
F32 = mybir.dt.float32
Exp = mybir.ActivationFunctionType.Exp
NPBF16 = ml_dtypes.bfloat16


# ---------------------------------------------------------------- host fits
_FIT_CACHE = {}


def _mlp_funcs(W1, b1, W2, b2, W3, b3, W4, b4):
    w1 = W1[:, 0].astype(np.float64)
    b1d, b2d, b3d = (b1.astype(np.float64), b2.astype(np.float64),
                     b3.astype(np.float64))
    W2d, W3d, W4d = (W2.astype(np.float64), W3.astype(np.float64),
                     W4.astype(np.float64))
    b4d = float(np.asarray(b4, np.float64)[0])

    def dF(x):
        x = np.ravel(x)
        h1 = np.outer(w1, x) + b1d[:, None]
        a1 = np.tanh(h1)
        d1 = (1 - a1 ** 2) * w1[:, None]
        h2 = W2d @ a1 + b2d[:, None]
        a2 = np.tanh(h2)
        d2 = (1 - a2 ** 2) * (W2d @ d1)
        h3 = W3d @ a2 + b3d[:, None]
        a3 = np.tanh(h3)
        d3 = (1 - a3 ** 2) * (W3d @ d2)
        return (W4d @ d3)[0]

    def F(x):
        x = np.ravel(x)
        h1 = np.tanh(np.outer(w1, x) + b1d[:, None])
        h2 = np.tanh(W2d @ h1 + b2d[:, None])
        h3 = np.tanh(W3d @ h2 + b3d[:, None])
        return (W4d @ h3)[0] + b4d

    return dF, F


def _ridge_fit(A, y, lam):
    cn = np.linalg.norm(A, axis=0)
    return np.linalg.solve(A.T @ A + lam * np.diag(cn ** 2), A.T @ y)


def _fits(W1, b1, W2, b2, W3, b3, W4, b4):
    key = b"".join(np.ascontiguousarray(a).tobytes()
                   for a in (W1, b1, W2, b2, W3, b3, W4, b4))
    if key in _FIT_CACHE:
        return _FIT_CACHE[key]
    dF, F = _mlp_funcs(W1, b1, W2, b2, W3, b3, W4, b4)
    F0 = float(F(np.zeros(1))[0])

    # dF: relative-weighted fit with absolute floor; scan geomspace rate grids
    gx = np.unique(np.concatenate([np.geomspace(1e-4, 100, 6000),
                                   np.linspace(0, 100, 6000)]))
    gy = dF(gx)
    wts = 1.0 / (np.abs(gy) + 2e-4)
    best = None
    for smin in (0.02, 0.04, 0.08, 0.15):
        for smax in (10.0, 13.0, 16.0, 20.0, 25.0):
            r = np.geomspace(smin, smax, K)
            A = np.exp(-np.outer(gx, r)) * wts[:, None]
            c = _ridge_fit(A, gy * wts, 1e-5)
            if np.abs(c).max() > 300.0:
                continue
            werr = np.abs((np.exp(-np.outer(gx, r)) @ c - gy) * wts).max()
            if best is None or werr < best[0]:
                best = (werr, r, c)
    _, rates, c = best

    # F - F0: absolute fit (const + exps), uniform grid
    gxF = np.linspace(0, 100, 20001)
    gyF = F(gxF) - F0
    ratesF = np.geomspace(0.04, 16.0, K)
    AF = np.concatenate([np.ones((len(gxF), 1)),
                         np.exp(-np.outer(gxF, ratesF))], axis=1)
    cf = _ridge_fit(AF, gyF, 1e-7)
    CF, cF = float(cf[0]), cf[1:]

    out = (rates, c, ratesF, cF, CF, F0)
    _FIT_CACHE[key] = out
    return out


# ---------------------------------------------------------------- packing
def _pack(t, lens):
    """-> xrow [NC, 16, XC] f32 (F cols first), seg_ev [NC, NTS*SEGT],
    (NTS, SEGT, FT), nF."""
    bs, ks = [], []
    for b in range(B):
        n = int(lens[b])
        ksb = np.arange(1, n, dtype=np.int64)
        ks.append(ksb)
        bs.append(np.full_like(ksb, b))
    bs = np.concatenate(bs)
    ks = np.concatenate(ks)
    nseg = (ks + SEG - 1) // SEG
    Tseg = int(nseg.sum())

    Spc = (Tseg + NC - 1) // NC
    NTS = (Spc + TS_MAX - 1) // TS_MAX
    SEGT = (Spc + NTS - 1) // NTS
    SEGT = (SEGT + 7) // 8 * 8
    Gc = NTS * SEGT                   # seg columns per core
    G = NC * Gc

    seg_b = np.zeros(G, np.int64)
    seg_k = np.zeros(G, np.int64)
    seg_j0 = np.zeros(G, np.int64)
    seg_ev = np.full(G, -1, np.int64)
    ev_idx = np.repeat(np.arange(len(ks)), nseg)
    seg_b[:Tseg] = bs[ev_idx]
    seg_k[:Tseg] = ks[ev_idx]
    seg_ev[:Tseg] = seg_b[:Tseg] * L + seg_k[:Tseg]
    starts = np.concatenate([[0], np.cumsum(nseg)[:-1]])
    seg_j0[:Tseg] = (np.arange(Tseg) - np.repeat(starts, nseg)) * SEG

    jj = seg_j0[:, None] + np.arange(SEG)[None, :]
    valid = jj < seg_k[:, None]
    valid[Tseg:] = False
    jc = np.minimum(jj, L - 1)
    x = np.where(valid,
                 (t[seg_b[:, None], jc] * -1.0 + t[seg_b, seg_k][:, None]),
                 XPAD).astype(np.float32)                  # [G, SEG]
    xs = x.reshape(NC, Gc, SEG).transpose(0, 2, 1)         # [NC, 16, Gc]

    # F points: one per valid event (all k < n)
    fb, fk = [], []
    for b in range(B):
        n = int(lens[b])
        fk.append(np.arange(n, dtype=np.int64))
        fb.append(np.full(n, b, np.int64))
    fb = np.concatenate(fb)
    fk = np.concatenate(fk)
    nF = len(fk)
    nFc = (nF + NC - 1) // NC
    FT = ((nFc + SEG - 1) // SEG + 1) // 2 * 2
    xf = np.full((NC * FT * SEG,), XPAD, np.float32)
    xf[:nF] = (T_END - t[fb, fk]).astype(np.float32)
    xf = xf.reshape(NC, FT, SEG).transpose(0, 2, 1)        # [NC, 16, FT]

    assert SEGT + FT <= TS_MAX
    xrow = np.concatenate([xf, xs], axis=2)                # [NC, 16, XC]
    return xrow, seg_ev.reshape(NC, Gc), (NTS, SEGT, FT), nF


# ---------------------------------------------------------------- program
_PROGRAM_CACHE = {}


def build_program(NTS, SEGT, FT):
    pkey = (NTS, SEGT, FT)
    if pkey in _PROGRAM_CACHE:
        return _PROGRAM_CACHE[pkey]
    XC = FT + NTS * SEGT
    NG = (NTS + GS - 1) // GS
    SELW = GS * GS + GS
    nc = bacc.Bacc("TRN2", target_bir_lowering=False, debug=False,
                   enable_asserts=False)

    xbb_d = nc.dram_tensor("xbb", [128, XC], BF16, kind="ExternalInput")
    selv_d = nc.dram_tensor("selv", [128, SELW], BF16, kind="ExternalInput")
    cf_d = nc.dram_tensor("cfd", [128, 4], F32, kind="ExternalInput")
    outs_d = nc.dram_tensor("out_s", [NG, GS, SEGT + FT], F32,
                            kind="ExternalOutput")

    # chunk boundaries in tiles; chunk 0 also carries the FT leading F cols
    bounds = [0, 1]
    while bounds[-1] < NTS:
        bounds.append(min(bounds[-1] + 2, NTS))
    NCH = len(bounds) - 1

    with tile.TileContext(nc) as tc, ExitStack() as ctx, \
            nc.allow_low_precision(reason="bf16 exp terms; tol is 2e-2"):
        consts = ctx.enter_context(tc.tile_pool(name="consts", bufs=1))
        xb_p = ctx.enter_context(tc.tile_pool(name="xb", bufs=NCH))
        term_p = ctx.enter_context(tc.tile_pool(name="term", bufs=3))
        outp_p = ctx.enter_context(tc.tile_pool(name="outp", bufs=2,
                                                space="PSUM"))
        stage_p = ctx.enter_context(tc.tile_pool(name="stage", bufs=2))

        # dummy ACT on a framework const AP: zero deps, so the compiler's
        # ACT_TABLE_LOAD for Exp runs during the input-DMA latency window
        zeros_ap = nc.const_aps.aps[(mybir.dt.float32, 0.0)]
        dummy = consts.tile([128, 1], F32, tag="dummy")
        nc.scalar.activation(dummy[:], zeros_ap, Exp)

        cf_raw = consts.tile([128, 4], F32, tag="cfraw")
        nc.gpsimd.dma_start(out=cf_raw[:], in_=cf_d.ap())
        sel_raw = consts.tile([128, SELW], BF16, tag="selraw")
        nc.gpsimd.dma_start(out=sel_raw[:], in_=selv_d.ap())
        cfc = consts.tile([128, 4], F32, tag="cfc")
        nc.vector.tensor_copy(cfc[:], cf_raw[:])
        selc = consts.tile([128, SELW], BF16, tag="selc")
        nc.vector.tensor_copy(selc[:], sel_raw[:])

        xbts = []
        for ci in range(NCH):
            a = FT + bounds[ci] * SEGT if ci > 0 else 0
            z = FT + bounds[ci + 1] * SEGT
            xbt = xb_p.tile([128, z - a], BF16, tag="xb")
            eng = nc.gpsimd if ci == NCH - 1 and NCH > 2 else nc.sync
            eng.dma_start(out=xbt[:], in_=xbb_d.ap()[:, a:z])
            xbts.append(xbt)

        # F chain first: its ACT/MM/copy/DMA never sit in the tail
        termf = term_p.tile([128, FT], BF16, tag="termf")
        nc.scalar.activation(termf[:], xbts[0][:, :FT], Exp,
                             bias=cfc[:, 3:4], scale=cfc[:, 2:3])

        terms = []
        for ci in range(NCH):
            a = FT if ci == 0 else 0
            tt = term_p.tile([128, (bounds[ci + 1] - bounds[ci]) * SEGT],
                             BF16, tag="terms")
            nc.scalar.activation(tt[:], xbts[ci][:, a:], Exp,
                                 bias=cfc[:, 1:2], scale=cfc[:, 0:1])
            terms.append((bounds[ci], tt))

        outbs = {}
        for ci, (t0, tt) in enumerate(terms):
            for j in range(bounds[ci + 1] - t0):
                tl = t0 + j
                g, tg = tl // GS, tl % GS
                in_g = min(GS, NTS - GS * g)
                if tg == 0:
                    w = SEGT + FT if g == 0 else SEGT
                    outbs[g] = outp_p.tile([GS, w], F32, tag="outb",
                                           name=f"outb{g}")
                    if g == 0:
                        nc.tensor.matmul(out=outbs[0][:, SEGT:],
                                         lhsT=selc[:, GS * GS:],
                                         rhs=termf[:], start=True, stop=True)
                nc.tensor.matmul(out=outbs[g][:, :SEGT],
                                 lhsT=selc[:, GS * tg:GS * tg + GS],
                                 rhs=tt[:, j * SEGT:(j + 1) * SEGT],
                                 start=(tg == 0), stop=(tg == in_g - 1))
                if tg == in_g - 1:
                    w = SEGT + FT if g == 0 else SEGT
                    st = stage_p.tile([GS, w], F32, tag="st", name=f"st{g}")
                    nc.vector.tensor_copy(st[:], outbs[g][:])
                    nc.gpsimd.dma_start(out=outs_d.ap()[g, :, :w], in_=st[:])

    nc.compile()
    prog = (nc, pkey)
    _PROGRAM_CACHE[pkey] = prog
    return prog


# ---------------------------------------------------------------- kernel
def _prepare(seq_pads, background, W1, b1, W2, b2, W3, b3, W4, b4, seq_lens):
    t = np.asarray(seq_pads)[:, :, 0].astype(np.float32)
    lens = np.asarray(seq_lens).astype(np.int64)
    rates, c, ratesF, cF, CF, F0 = _fits(
        np.asarray(W1, np.float64), np.asarray(b1, np.float64),
        np.asarray(W2, np.float64), np.asarray(b2, np.float64),
        np.asarray(W3, np.float64), np.asarray(b3, np.float64),
        np.asarray(W4, np.float64), np.asarray(b4, np.float64))

    xrow, seg_ev, (NTS, SEGT, FT), nF = _pack(t, lens)
    nc, _ = build_program(NTS, SEGT, FT)

    grp = np.repeat(np.arange(K), SEG)                     # partition -> rate
    cfd = np.zeros((128, 4), np.float32)
    cfd[:, 0] = -rates[grp]
    cfd[:, 1] = np.log(np.maximum(np.abs(c), 1e-20))[grp]
    cfd[:, 2] = -ratesF[grp]
    cfd[:, 3] = np.log(np.maximum(np.abs(cF), 1e-20))[grp]
    selv = np.zeros((128, GS * GS + GS), np.float32)
    sgn = np.sign(c)[grp]
    for v in range(GS):
        selv[:, GS * v + v] = sgn
    selv[:, GS * GS] = np.sign(cF)[grp]
    selv = selv.astype(NPBF16)

    in_maps = []
    for cix in range(NC):
        m = dict(selv=selv, cfd=cfd)
        m["xbb"] = np.ascontiguousarray(
            np.tile(xrow[cix], (K, 1)).astype(NPBF16))
        in_maps.append(m)
    meta = dict(seg_ev=seg_ev, NTS=NTS, SEGT=SEGT, FT=FT, nF=nF, CF=CF,
                lens=lens)
    return nc, in_maps, meta


def kernel(seq_pads, background, W1, b1, W2, b2, W3, b3, W4, b4, seq_lens):
    nc, in_maps, meta = _prepare(seq_pads, background, W1, b1, W2, b2, W3,
                                 b3, W4, b4, seq_lens)
    NTS, SEGT, FT = meta["NTS"], meta["SEGT"], meta["FT"]
    NG = (NTS + GS - 1) // GS
    lens = meta["lens"]

    def valid_parts(res):
        outs = []
        for cc in range(NC):
            o = res[cc]["out_s"]                           # [NG, GS, SEGT+FT]
            segs = []
            for g in range(NG):
                in_g = min(GS, NTS - GS * g)
                segs.append(o[g, :in_g, :SEGT].reshape(-1))
            outs.append((np.concatenate(segs), o[0, :, SEGT:]))
        return outs

    res = run_bass_kernel_spmd(nc, in_maps, core_ids=list(range(NC))).results
    vp = valid_parts(res)
    if any(not (np.isfinite(a).all() and np.isfinite(b).all())
           for a, b in vp):
        res = run_bass_kernel_spmd(nc, in_maps,
                                   core_ids=list(range(NC))).results
        vp = valid_parts(res)

    Gc = NTS * SEGT
    partials = np.concatenate([a[:Gc] for a, _ in vp])
    f_exp = float(sum(b.sum() for _, b in vp))

    S = np.zeros(B * L, np.float64)
    seg_ev = meta["seg_ev"].reshape(-1)
    ok = seg_ev >= 0
    np.add.at(S, seg_ev[ok], partials[ok].astype(np.float64))
    S = S.reshape(B, L)

    bg = float(np.asarray(background)[0])
    lam = bg + S
    mask = np.arange(L)[None, :] < lens[:, None]
    sum_log = np.log(np.where(mask, lam, 1.0)).sum()

    ints_total = f_exp + meta["nF"] * meta["CF"] + B * T_END * bg
    nll = -(sum_log - ints_total) / B
    return np.float32(nll)


# revision 9
# speedup vs baseline: 1.1182x; 1.1182x over previous
"""Trainium2 Bass kernel for nn_AutoIntTPPSameInfluence — exp-sum formulation.

dF(x) (the scalar derivative of the 1->64->64->64->1 tanh MLP) is fit on host
as a sum of 8 decaying exponentials  dF(x) ~= sum_i c_i exp(-s_i x)  (ridge
LSQ over a geomspace rate grid).  On device every pairwise influence point is
then ONE table-exp evaluation: points are packed 16-per-segment-column and
replicated over the 8 partition groups of a [128, N] tile, a single ACT
instruction computes |c_i| exp(-s_i x + ln|c_i|) for all 8 rates via
per-partition scale/bias, and one bf16 matmul with a +-1 selector contracts
all 128 partitions — performing the 16-point segment sum AND the 8-term
weighted sum at 1 col/cycle.  Masked/padded points use x = 3e4, driving every
exponential to exactly 0.

The integral term sum_k F(T_END - t_k) - F0 only needs a bulk sum, so
F(x) - F0 is fit the same way (const + 8 exps) and rides through the same
pipeline as the leading FT columns; the constant is applied on host via the
valid count.  Host: scatter segment sums to events, log/mask/reduce in f64.

Schedule notes: a dummy ACT on a framework const AP forces the exp
ACT_TABLE_LOAD to run during the input-DMA completion window (~2.5 us fixed
DMA latency); F columns lead so their chain never sits in the tail; PSUM
accumulation groups of 4 tiles let output DMAs start mid-run.
"""

import numpy as np
from contextlib import ExitStack

import ml_dtypes

import concourse.bass as bass
import concourse.bacc as bacc
import concourse.tile as tile
import concourse.mybir as mybir
from concourse.bass_utils import run_bass_kernel_spmd

B, L, H = 16, 320, 64
T_END = 100.0
NC = 8
SEG = 16                    # points per segment column
K = 8                       # exponential rates (K * SEG == 128)
GS = 4                      # tiles per PSUM accumulation group
TS_MAX = 512                # PSUM bank width in f32
XPAD = np.float32(30000.0)  # pad x: exp(-s*XPAD) underflows to exactly 0
BF16 = mybir.dt.bfloat16
F32 = mybir.dt.float32
Exp = mybir.ActivationFunctionType.Exp
NPBF16 = ml_dtypes.bfloat16


# ---------------------------------------------------------------- host fits
_FIT_CACHE = {}


def _mlp_funcs(W1, b1, W2, b2, W3, b3, W4, b4):
    w1 = W1[:, 0].astype(np.float64)
    b1d, b2d, b3d = (b1.astype(np.float64), b2.astype(np.float64),
                     b3.astype(np.float64))
    W2d, W3d, W4d = (W2.astype(np.float64), W3.astype(np.float64),
                     W4.astype(np.float64))
    b4d = float(np.asarray(b4, np.float64)[0])

    def dF(x):
        x = np.ravel(x)
        h1 = np.outer(w1, x) + b1d[:, None]
        a1 = np.tanh(h1)
        d1 = (1 - a1 ** 2) * w1[:, None]
        h2 = W2d @ a1 + b2d[:, None]
        a2 = np.tanh(h2)
        d2 = (1 - a2 ** 2) * (W2d @ d1)
        h3 = W3d @ a2 + b3d[:, None]
        a3 = np.tanh(h3)
        d3 = (1 - a3 ** 2) * (W3d @ d2)
        return (W4d @ d3)[0]

    def F(x):
        x = np.ravel(x)
        h1 = np.tanh(np.outer(w1, x) + b1d[:, None])
        h2 = np.tanh(W2d @ h1 + b2d[:, None])
        h3 = np.tanh(W3d @ h2 + b3d[:, None])
        return (W4d @ h3)[0] + b4d

    return dF, F


def _ridge_fit(A, y, lam):
    cn = np.linalg.norm(A, axis=0)
    return np.linalg.solve(A.T @ A + lam * np.diag(cn ** 2), A.T @ y)


def _fits(W1, b1, W2, b2, W3, b3, W4, b4):
    key = b"".join(np.ascontiguousarray(a).tobytes()
                   for a in (W1, b1, W2, b2, W3, b3, W4, b4))
    if key in _FIT_CACHE:
        return _FIT_CACHE[key]
    dF, F = _mlp_funcs(W1, b1, W2, b2, W3, b3, W4, b4)
    F0 = float(F(np.zeros(1))[0])

    # dF: relative-weighted fit with absolute floor; scan geomspace rate grids
    gx = np.unique(np.concatenate([np.geomspace(1e-4, 100, 6000),
                                   np.linspace(0, 100, 6000)]))
    gy = dF(gx)
    wts = 1.0 / (np.abs(gy) + 2e-4)
    best = None
    for smin in (0.02, 0.04, 0.08, 0.15):
        for smax in (10.0, 13.0, 16.0, 20.0, 25.0):
            r = np.geomspace(smin, smax, K)
            A = np.exp(-np.outer(gx, r)) * wts[:, None]
            c = _ridge_fit(A, gy * wts, 1e-5)
            if np.abs(c).max() > 300.0:
                continue
            werr = np.abs((np.exp(-np.outer(gx, r)) @ c - gy) * wts).max()
            if best is None or werr < best[0]:
                best = (werr, r, c)
    _, rates, c = best

    # F - F0: absolute fit (const + exps), uniform grid
    gxF = np.linspace(0, 100, 20001)
    gyF = F(gxF) - F0
    ratesF = np.geomspace(0.04, 16.0, K)
    AF = np.concatenate([np.ones((len(gxF), 1)),
                         np.exp(-np.outer(gxF, ratesF))], axis=1)
    cf = _ridge_fit(AF, gyF, 1e-7)
    CF, cF = float(cf[0]), cf[1:]

    out = (rates, c, ratesF, cF, CF, F0)
    _FIT_CACHE[key] = out
    return out


# ---------------------------------------------------------------- packing
def _pack(t, lens):
    """-> xrow [NC, 16, XC] f32 (F cols first), seg_ev [NC, NTS*SEGT],
    (NTS, SEGT, FT), nF."""
    bs, ks = [], []
    for b in range(B):
        n = int(lens[b])
        ksb = np.arange(1, n, dtype=np.int64)
        ks.append(ksb)
        bs.append(np.full_like(ksb, b))
    bs = np.concatenate(bs)
    ks = np.concatenate(ks)
    nseg = (ks + SEG - 1) // SEG
    Tseg = int(nseg.sum())

    Spc = (Tseg + NC - 1) // NC
    NTS = (Spc + TS_MAX - 1) // TS_MAX
    SEGT = (Spc + NTS - 1) // NTS
    SEGT = (SEGT + 7) // 8 * 8
    Gc = NTS * SEGT                   # seg columns per core
    G = NC * Gc

    seg_b = np.zeros(G, np.int64)
    seg_k = np.zeros(G, np.int64)
    seg_j0 = np.zeros(G, np.int64)
    seg_ev = np.full(G, -1, np.int64)
    ev_idx = np.repeat(np.arange(len(ks)), nseg)
    seg_b[:Tseg] = bs[ev_idx]
    seg_k[:Tseg] = ks[ev_idx]
    seg_ev[:Tseg] = seg_b[:Tseg] * L + seg_k[:Tseg]
    starts = np.concatenate([[0], np.cumsum(nseg)[:-1]])
    seg_j0[:Tseg] = (np.arange(Tseg) - np.repeat(starts, nseg)) * SEG

    jj = seg_j0[:, None] + np.arange(SEG)[None, :]
    valid = jj < seg_k[:, None]
    valid[Tseg:] = False
    jc = np.minimum(jj, L - 1)
    x = np.where(valid,
                 (t[seg_b[:, None], jc] * -1.0 + t[seg_b, seg_k][:, None]),
                 XPAD).astype(np.float32)                  # [G, SEG]
    xs = x.reshape(NC, Gc, SEG).transpose(0, 2, 1)         # [NC, 16, Gc]

    # F points: one per valid event (all k < n)
    fb, fk = [], []
    for b in range(B):
        n = int(lens[b])
        fk.append(np.arange(n, dtype=np.int64))
        fb.append(np.full(n, b, np.int64))
    fb = np.concatenate(fb)
    fk = np.concatenate(fk)
    nF = len(fk)
    nFc = (nF + NC - 1) // NC
    FT = ((nFc + SEG - 1) // SEG + 1) // 2 * 2
    xf = np.full((NC * FT * SEG,), XPAD, np.float32)
    xf[:nF] = (T_END - t[fb, fk]).astype(np.float32)
    xf = xf.reshape(NC, FT, SEG).transpose(0, 2, 1)        # [NC, 16, FT]

    assert SEGT + FT <= TS_MAX
    xrow = np.concatenate([xf, xs], axis=2)                # [NC, 16, XC]
    return xrow, seg_ev.reshape(NC, Gc), (NTS, SEGT, FT), nF


# ---------------------------------------------------------------- program
_PROGRAM_CACHE = {}


def build_program(NTS, SEGT, FT):
    pkey = (NTS, SEGT, FT)
    if pkey in _PROGRAM_CACHE:
        return _PROGRAM_CACHE[pkey]
    XC = FT + NTS * SEGT
    NG = (NTS + GS - 1) // GS
    SELW = GS * GS + GS
    nc = bacc.Bacc("TRN2", target_bir_lowering=False, debug=False,
                   enable_asserts=False)

    xbb_d = nc.dram_tensor("xbb", [128, XC], BF16, kind="ExternalInput")
    selv_d = nc.dram_tensor("selv", [128, SELW], BF16, kind="ExternalInput")
    cf_d = nc.dram_tensor("cfd", [128, 4], F32, kind="ExternalInput")
    outs_d = nc.dram_tensor("out_s", [NG, GS, SEGT + FT], F32,
                            kind="ExternalOutput")

    # chunk boundaries in tiles; chunk 0 also carries the FT leading F cols
    bounds = [0, 1]
    while bounds[-1] < NTS:
        bounds.append(min(bounds[-1] + 2, NTS))
    NCH = len(bounds) - 1

    with tile.TileContext(nc) as tc, ExitStack() as ctx, \
            nc.allow_low_precision(reason="bf16 exp terms; tol is 2e-2"):
        consts = ctx.enter_context(tc.tile_pool(name="consts", bufs=1))
        xb_p = ctx.enter_context(tc.tile_pool(name="xb", bufs=NCH))
        term_p = ctx.enter_context(tc.tile_pool(name="term", bufs=3))
        outp_p = ctx.enter_context(tc.tile_pool(name="outp", bufs=2,
                                                space="PSUM"))
        stage_p = ctx.enter_context(tc.tile_pool(name="stage", bufs=2))

        # dummy ACT on a framework const AP: zero deps, so the compiler's
        # ACT_TABLE_LOAD for Exp runs during the input-DMA latency window
        zeros_ap = nc.const_aps.aps[(mybir.dt.float32, 0.0)]
        dummy = consts.tile([128, 1], F32, tag="dummy")
        nc.scalar.activation(dummy[:], zeros_ap, Exp)

        cfc = consts.tile([128, 4], F32, tag="cfc")
        nc.sync.dma_start(out=cfc[:], in_=cf_d.ap())
        selc = consts.tile([128, SELW], BF16, tag="selc")
        nc.sync.dma_start(out=selc[:], in_=selv_d.ap())

        xbts = []
        for ci in range(NCH):
            a = FT + bounds[ci] * SEGT if ci > 0 else 0
            z = FT + bounds[ci + 1] * SEGT
            xbt = xb_p.tile([128, z - a], BF16, tag="xb")
            eng = nc.gpsimd if ci == NCH - 1 and NCH > 2 else nc.sync
            eng.dma_start(out=xbt[:], in_=xbb_d.ap()[:, a:z])
            xbts.append(xbt)

        # F chain first: its ACT/MM/copy/DMA never sit in the tail
        termf = term_p.tile([128, FT], BF16, tag="termf")
        nc.scalar.activation(termf[:], xbts[0][:, :FT], Exp,
                             bias=cfc[:, 3:4], scale=cfc[:, 2:3])

        terms = []
        for ci in range(NCH):
            a = FT if ci == 0 else 0
            tt = term_p.tile([128, (bounds[ci + 1] - bounds[ci]) * SEGT],
                             BF16, tag="terms")
            nc.scalar.activation(tt[:], xbts[ci][:, a:], Exp,
                                 bias=cfc[:, 1:2], scale=cfc[:, 0:1])
            terms.append((bounds[ci], tt))

        outbs = {}
        for ci, (t0, tt) in enumerate(terms):
            for j in range(bounds[ci + 1] - t0):
                tl = t0 + j
                g, tg = tl // GS, tl % GS
                in_g = min(GS, NTS - GS * g)
                if tg == 0:
                    w = SEGT + FT if g == 0 else SEGT
                    outbs[g] = outp_p.tile([GS, w], F32, tag="outb",
                                           name=f"outb{g}")
                    if g == 0:
                        nc.tensor.matmul(out=outbs[0][:, SEGT:],
                                         lhsT=selc[:, GS * GS:],
                                         rhs=termf[:], start=True, stop=True)
                nc.tensor.matmul(out=outbs[g][:, :SEGT],
                                 lhsT=selc[:, GS * tg:GS * tg + GS],
                                 rhs=tt[:, j * SEGT:(j + 1) * SEGT],
                                 start=(tg == 0), stop=(tg == in_g - 1))
                if tg == in_g - 1:
                    w = SEGT + FT if g == 0 else SEGT
                    st = stage_p.tile([GS, w], F32, tag="st", name=f"st{g}")
                    nc.vector.tensor_copy(st[:], outbs[g][:])
                    nc.gpsimd.dma_start(out=outs_d.ap()[g, :, :w], in_=st[:])

    nc.compile()
    prog = (nc, pkey)
    _PROGRAM_CACHE[pkey] = prog
    return prog


# ---------------------------------------------------------------- kernel
def _prepare(seq_pads, background, W1, b1, W2, b2, W3, b3, W4, b4, seq_lens):
    t = np.asarray(seq_pads)[:, :, 0].astype(np.float32)
    lens = np.asarray(seq_lens).astype(np.int64)
    rates, c, ratesF, cF, CF, F0 = _fits(
        np.asarray(W1, np.float64), np.asarray(b1, np.float64),
        np.asarray(W2, np.float64), np.asarray(b2, np.float64),
        np.asarray(W3, np.float64), np.asarray(b3, np.float64),
        np.asarray(W4, np.float64), np.asarray(b4, np.float64))

    xrow, seg_ev, (NTS, SEGT, FT), nF = _pack(t, lens)
    nc, _ = build_program(NTS, SEGT, FT)

    grp = np.repeat(np.arange(K), SEG)                     # partition -> rate
    cfd = np.zeros((128, 4), np.float32)
    cfd[:, 0] = -rates[grp]
    cfd[:, 1] = np.log(np.maximum(np.abs(c), 1e-20))[grp]
    cfd[:, 2] = -ratesF[grp]
    cfd[:, 3] = np.log(np.maximum(np.abs(cF), 1e-20))[grp]
    selv = np.zeros((128, GS * GS + GS), np.float32)
    sgn = np.sign(c)[grp]
    for v in range(GS):
        selv[:, GS * v + v] = sgn
    selv[:, GS * GS] = np.sign(cF)[grp]
    selv = selv.astype(NPBF16)

    in_maps = []
    for cix in range(NC):
        m = dict(selv=selv, cfd=cfd)
        m["xbb"] = np.ascontiguousarray(
            np.tile(xrow[cix], (K, 1)).astype(NPBF16))
        in_maps.append(m)
    meta = dict(seg_ev=seg_ev, NTS=NTS, SEGT=SEGT, FT=FT, nF=nF, CF=CF,
                lens=lens)
    return nc, in_maps, meta


def kernel(seq_pads, background, W1, b1, W2, b2, W3, b3, W4, b4, seq_lens):
    nc, in_maps, meta = _prepare(seq_pads, background, W1, b1, W2, b2, W3,
                                 b3, W4, b4, seq_lens)
    NTS, SEGT, FT = meta["NTS"], meta["SEGT"], meta["FT"]
    NG = (NTS + GS - 1) // GS
    lens = meta["lens"]

    def valid_parts(res):
        outs = []
        for cc in range(NC):
            o = res[cc]["out_s"]                           # [NG, GS, SEGT+FT]
            segs = []
            for g in range(NG):
                in_g = min(GS, NTS - GS * g)
                segs.append(o[g, :in_g, :SEGT].reshape(-1))
            outs.append((np.concatenate(segs), o[0, :, SEGT:]))
        return outs

    res = run_bass_kernel_spmd(nc, in_maps, core_ids=list(range(NC))).results
    vp = valid_parts(res)
    if any(not (np.isfinite(a).all() and np.isfinite(b).all())
           for a, b in vp):
        res = run_bass_kernel_spmd(nc, in_maps,
                                   core_ids=list(range(NC))).results
        vp = valid_parts(res)

    Gc = NTS * SEGT
    partials = np.concatenate([a[:Gc] for a, _ in vp])
    f_exp = float(sum(b.sum() for _, b in vp))

    S = np.zeros(B * L, np.float64)
    seg_ev = meta["seg_ev"].reshape(-1)
    ok = seg_ev >= 0
    np.add.at(S, seg_ev[ok], partials[ok].astype(np.float64))
    S = S.reshape(B, L)

    bg = float(np.asarray(background)[0])
    lam = bg + S
    mask = np.arange(L)[None, :] < lens[:, None]
    sum_log = np.log(np.where(mask, lam, 1.0)).sum()

    ints_total = f_exp + meta["nF"] * meta["CF"] + B * T_END * bg
    nll = -(sum_log - ints_total) / B
    return np.float32(nll)


# revision 12
# speedup vs baseline: 1.2467x; 1.1149x over previous
"""Trainium2 Bass kernel for nn_AutoIntTPPSameInfluence — exp-sum formulation.

dF(x) (the scalar derivative of the 1->64->64->64->1 tanh MLP) is fit on host
as sums of decaying exponentials, split by range:
  near (x < X0):   8 rates, 16 points per segment column (8*16 = 128)
  far (X0<=x<W):   4 rates, 32 points per segment column (4*32 = 128)
  x >= W:          dropped (dF tail integral < 1e-4 -> per-lambda error ~3e-4)
On device every pairwise influence point is ONE table-exp evaluation: points
are packed SEG-per-column and replicated over the partition groups of a
[128, N] tile, a single ACT instruction computes |c_i| exp(-s_i x + ln|c_i|)
for all rates via per-partition scale/bias, and one bf16 matmul with a +-1
selector contracts all 128 partitions — performing the SEG-point segment sum
AND the weighted rate sum at 1 col/cycle.  Masked/padded points use x = 3e4,
driving every exponential to exactly 0.

The integral term sum_k F(T_END - t_k) - F0 only needs a bulk sum, so
F(x) - F0 is fit the same way (const + 8 exps) and rides through the same
pipeline as the leading FT columns; the constant is applied on host via the
valid count.  Host: scatter segment sums to events, log/mask/reduce in f64.

Schedule notes: a dummy ACT on a framework const AP forces the exp
ACT_TABLE_LOAD to run during the input-DMA completion window (~2.5 us fixed
DMA latency); const DMAs go first on the sync queue and are consumed without
staging copies; F columns lead so their chain never sits in the tail; PSUM
accumulation groups let output DMAs start mid-run.
"""

import numpy as np
from contextlib import ExitStack

import ml_dtypes

import concourse.bass as bass
import concourse.bacc as bacc
import concourse.tile as tile
import concourse.mybir as mybir
from concourse.bass_utils import run_bass_kernel_spmd

B, L, H = 16, 320, 64
T_END = 100.0
NC = 8
SEGN = 16                   # near: points per column, 8 rates
SEGR = 32                   # far: points per column, 4 rates
X0 = 4.0                    # near/far boundary
WCUT = 12.0                 # truncation window
GS = 4                      # tiles per PSUM accumulation group
TS_MAX = 512                # PSUM bank width in f32
XPAD = np.float32(30000.0)  # pad x: exp(-s*XPAD) underflows to exactly 0
BF16 = mybir.dt.bfloat16
F32 = mybir.dt.float32
Exp = mybir.ActivationFunctionType.Exp
NPBF16 = ml_dtypes.bfloat16


# ---------------------------------------------------------------- host fits
_FIT_CACHE = {}


def _mlp_funcs(W1, b1, W2, b2, W3, b3, W4, b4):
    w1 = W1[:, 0].astype(np.float64)
    b1d, b2d, b3d = (b1.astype(np.float64), b2.astype(np.float64),
                     b3.astype(np.float64))
    W2d, W3d, W4d = (W2.astype(np.float64), W3.astype(np.float64),
                     W4.astype(np.float64))
    b4d = float(np.asarray(b4, np.float64)[0])

    def dF(x):
        x = np.ravel(x)
        h1 = np.outer(w1, x) + b1d[:, None]
        a1 = np.tanh(h1)
        d1 = (1 - a1 ** 2) * w1[:, None]
        h2 = W2d @ a1 + b2d[:, None]
        a2 = np.tanh(h2)
        d2 = (1 - a2 ** 2) * (W2d @ d1)
        h3 = W3d @ a2 + b3d[:, None]
        a3 = np.tanh(h3)
        d3 = (1 - a3 ** 2) * (W3d @ d2)
        return (W4d @ d3)[0]

    def F(x):
        x = np.ravel(x)
        h1 = np.tanh(np.outer(w1, x) + b1d[:, None])
        h2 = np.tanh(W2d @ h1 + b2d[:, None])
        h3 = np.tanh(W3d @ h2 + b3d[:, None])
        return (W4d @ h3)[0] + b4d

    return dF, F


def _ridge_fit(A, y, lam):
    cn = np.linalg.norm(A, axis=0)
    return np.linalg.solve(A.T @ A + lam * np.diag(cn ** 2), A.T @ y)


def _scan_fit(gx, gy, floor, nk, smins, smaxs, cmax):
    wts = 1.0 / (np.abs(gy) + floor)
    best = None
    for smin in smins:
        for smax in smaxs:
            r = np.geomspace(smin, smax, nk)
            A = np.exp(-np.outer(gx, r)) * wts[:, None]
            c = _ridge_fit(A, gy * wts, 1e-5)
            if np.abs(c).max() > cmax:
                continue
            werr = np.abs((np.exp(-np.outer(gx, r)) @ c - gy) * wts).max()
            if best is None or werr < best[0]:
                best = (werr, r, c)
    return best[1], best[2]


def _fits(W1, b1, W2, b2, W3, b3, W4, b4):
    key = b"".join(np.ascontiguousarray(a).tobytes()
                   for a in (W1, b1, W2, b2, W3, b3, W4, b4))
    if key in _FIT_CACHE:
        return _FIT_CACHE[key]
    dF, F = _mlp_funcs(W1, b1, W2, b2, W3, b3, W4, b4)
    F0 = float(F(np.zeros(1))[0])

    gx = np.unique(np.concatenate([np.geomspace(1e-4, X0 * 1.02, 4000),
                                   np.linspace(0, X0 * 1.02, 4000)]))
    rn, cn = _scan_fit(gx, dF(gx), 1e-4, 8,
                       (0.3, 0.5, 0.8, 1.2), (10., 14., 18., 24., 30.), 300.)
    gxf = np.linspace(X0 * 0.98, WCUT * 1.02, 6000)
    rf, cf = _scan_fit(gxf, dF(gxf), 2e-6, 4,
                       (0.1, 0.2, 0.3, 0.5), (1.0, 1.5, 2.5, 4.0), 1e3)

    gxF = np.linspace(0, 100, 20001)
    gyF = F(gxF) - F0
    rF = np.geomspace(0.04, 16.0, 8)
    AF = np.concatenate([np.ones((len(gxF), 1)),
                         np.exp(-np.outer(gxF, rF))], axis=1)
    cfF = _ridge_fit(AF, gyF, 1e-7)
    CF, cF = float(cfF[0]), cfF[1:]

    out = dict(rn=rn, cn=cn, rf=rf, cf=cf, rF=rF, cF=cF, CF=CF, F0=F0)
    _FIT_CACHE[key] = out
    return out


# ---------------------------------------------------------------- packing
def _seg_stream(t, lens, lo_f, hi_f, seg):
    """Segments of `seg` consecutive j's with t_k - t_j in [lo, hi) per event.
    Returns x [Tseg, seg] f32 (XPAD-padded), ev [Tseg] (b*L + k)."""
    xs, evs = [], []
    for b in range(B):
        n = int(lens[b])
        tb = t[b, :n].astype(np.float64)
        jhi = np.searchsorted(tb, tb - lo_f) if lo_f > 0 else np.arange(n)
        jlo = np.searchsorted(tb, tb - hi_f) if hi_f is not None else \
            np.zeros(n, np.int64)
        jhi = np.minimum(jhi, np.arange(n))
        cnt = jhi - jlo
        nsg = (cnt + seg - 1) // seg
        ev_idx = np.repeat(np.arange(n), nsg)
        starts = np.concatenate([[0], np.cumsum(nsg)[:-1]])
        within = (np.arange(int(nsg.sum())) - np.repeat(starts, nsg)) * seg
        j0 = jlo[ev_idx] + within
        jj = j0[:, None] + np.arange(seg)[None, :]
        valid = jj < jhi[ev_idx][:, None]
        jc = np.minimum(jj, n - 1)
        x = np.where(valid, tb[ev_idx][:, None] * 0 + (t[b, ev_idx][:, None]
                     - tb[jc]), XPAD).astype(np.float32)
        xs.append(x)
        evs.append(b * L + ev_idx)
    return np.concatenate(xs), np.concatenate(evs)


def _layout_stream(x, ev, seg, nrep):
    """Split stream across cores; -> xcore [NC, 128, cols_pc], ev [NC, cols],
    (NT, SEGT)."""
    Tseg = x.shape[0]
    Spc = (Tseg + NC - 1) // NC
    NT = (Spc + TS_MAX - 1) // TS_MAX
    SEGT = ((Spc + NT - 1) // NT + 7) // 8 * 8
    cap = NC * NT * SEGT
    xp = np.full((cap, seg), XPAD, np.float32)
    xp[:Tseg] = x
    evp = np.full(cap, -1, np.int64)
    evp[:Tseg] = ev
    xc = xp.reshape(NC, NT * SEGT, seg).transpose(0, 2, 1)  # [NC, seg, cols]
    xc = np.tile(xc, (1, nrep, 1))                          # [NC, 128, cols]
    return xc, evp.reshape(NC, NT * SEGT), NT, SEGT


def _pack(t, lens):
    xn, evn = _seg_stream(t, lens, 0.0, X0, SEGN)
    xr, evr = _seg_stream(t, lens, X0, WCUT, SEGR)
    xcn, evn, TN, SEGTN = _layout_stream(xn, evn, SEGN, 128 // SEGN)
    xcr, evr, TR, SEGTR = _layout_stream(xr, evr, SEGR, 128 // SEGR)

    # F points: one per valid event (all k < n)
    fx = []
    for b in range(B):
        n = int(lens[b])
        fx.append((T_END - t[b, :n]).astype(np.float32))
    fx = np.concatenate(fx)
    nF = len(fx)
    nFc = (nF + NC - 1) // NC
    FT = ((nFc + SEGN - 1) // SEGN + 1) // 2 * 2
    xf = np.full((NC * FT * SEGN,), XPAD, np.float32)
    xf[:nF] = fx
    xf = xf.reshape(NC, FT, SEGN).transpose(0, 2, 1)
    xf = np.tile(xf, (1, 128 // SEGN, 1))                   # [NC, 128, FT]

    assert SEGTN + FT <= TS_MAX
    xbb = np.concatenate([xf, xcn, xcr], axis=2)            # [NC, 128, XC]
    return xbb, (evn, evr), (TN, SEGTN, TR, SEGTR, FT), nF


# ---------------------------------------------------------------- program
_PROGRAM_CACHE = {}


def build_program(TN, SEGTN, TR, SEGTR, FT):
    pkey = (TN, SEGTN, TR, SEGTR, FT)
    if pkey in _PROGRAM_CACHE:
        return _PROGRAM_CACHE[pkey]
    XC = FT + TN * SEGTN + TR * SEGTR
    NGn = (TN + GS - 1) // GS
    NGr = (TR + GS - 1) // GS
    OUTW = max(SEGTN + FT, SEGTR)
    SELW = GS * GS * 2 + GS
    nc = bacc.Bacc("TRN2", target_bir_lowering=False, debug=False,
                   enable_asserts=False)

    xbb_d = nc.dram_tensor("xbb", [128, XC], BF16, kind="ExternalInput")
    selv_d = nc.dram_tensor("selv", [128, SELW], BF16, kind="ExternalInput")
    cf_d = nc.dram_tensor("cfd", [128, 6], F32, kind="ExternalInput")
    outs_d = nc.dram_tensor("out_s", [NGn + NGr, GS, OUTW], F32,
                            kind="ExternalOutput")

    # tiles: ("N", i) near, ("R", i) far; chunks pair tiles within a stream
    tiles = [("N", i) for i in range(TN)] + [("R", i) for i in range(TR)]
    chunks = []
    i = 0
    while i < len(tiles):
        if i + 1 < len(tiles) and tiles[i][0] == tiles[i + 1][0]:
            chunks.append(tiles[i:i + 2])
            i += 2
        else:
            chunks.append(tiles[i:i + 1])
            i += 1

    def col0(tl):
        s, i = tl
        return FT + i * SEGTN if s == "N" else FT + TN * SEGTN + i * SEGTR

    def width(tl):
        return SEGTN if tl[0] == "N" else SEGTR

    with tile.TileContext(nc) as tc, ExitStack() as ctx, \
            nc.allow_low_precision(reason="bf16 exp terms; tol is 2e-2"):
        consts = ctx.enter_context(tc.tile_pool(name="consts", bufs=1))
        xb_p = ctx.enter_context(tc.tile_pool(name="xb", bufs=len(chunks)))
        term_p = ctx.enter_context(tc.tile_pool(name="term", bufs=3))
        outp_p = ctx.enter_context(tc.tile_pool(name="outp", bufs=4,
                                                space="PSUM"))
        stage_p = ctx.enter_context(tc.tile_pool(name="stage", bufs=2))

        # const DMAs ride the scalar queue ahead of the dummy ACT, so the
        # compiler-inserted ACT_TABLE_LOAD for Exp (bound to the dummy, which
        # has zero data deps) overlaps their ~2.5 us completion latency
        cfc = consts.tile([128, 6], F32, tag="cfc")
        nc.scalar.dma_start(out=cfc[:], in_=cf_d.ap())
        selc = consts.tile([128, SELW], BF16, tag="selc")
        nc.scalar.dma_start(out=selc[:], in_=selv_d.ap())
        zeros_ap = nc.const_aps.aps[(mybir.dt.float32, 0.0)]
        dummy = consts.tile([128, 1], F32, tag="dummy")
        nc.scalar.activation(dummy[:], zeros_ap, Exp)

        xbts = []
        for ci, ch in enumerate(chunks):
            a = 0 if ci == 0 else col0(ch[0])
            z = col0(ch[-1]) + width(ch[-1])
            xbt = xb_p.tile([128, z - a], BF16, tag="xb", name=f"xb{ci}")
            eng = nc.gpsimd if ci == len(chunks) - 1 and len(chunks) > 2 \
                else nc.sync
            eng.dma_start(out=xbt[:], in_=xbb_d.ap()[:, a:z])
            xbts.append((a, xbt))

        # F chain first: its ACT/MM/copy/DMA never sit in the tail
        termf = term_p.tile([128, FT], BF16, tag="termf")
        nc.scalar.activation(termf[:], xbts[0][1][:, :FT], Exp,
                             bias=cfc[:, 3:4], scale=cfc[:, 2:3])

        cpair = {"N": (cfc[:, 0:1], cfc[:, 1:2]),
                 "R": (cfc[:, 5:6], cfc[:, 4:5])}
        sblk = {"N": 0, "R": GS * GS}

        outbs, terms = {}, []
        for ci, ch in enumerate(chunks):
            a, xbt = xbts[ci]
            c0, c1 = col0(ch[0]) - a, col0(ch[-1]) + width(ch[-1]) - a
            tt = term_p.tile([128, c1 - c0], BF16, tag="terms",
                             name=f"terms{ci}")
            sc, bi = cpair[ch[0][0]]
            nc.scalar.activation(tt[:], xbt[:, c0:c1], Exp, bias=bi, scale=sc)
            for j, tl in enumerate(ch):
                s, i = tl
                w = width(tl)
                g = i // GS + (0 if s == "N" else NGn)
                tg = i % GS
                in_g = min(GS, (TN if s == "N" else TR) - GS * (i // GS))
                if tg == 0:
                    ow = SEGTN + FT if g == 0 else w
                    outbs[g] = outp_p.tile([GS, ow], F32, tag="outb",
                                           name=f"outb{g}")
                    if g == 0:
                        nc.tensor.matmul(out=outbs[0][:, SEGTN:],
                                         lhsT=selc[:, 2 * GS * GS:],
                                         rhs=termf[:], start=True, stop=True)
                nc.tensor.matmul(out=outbs[g][:, :w],
                                 lhsT=selc[:, sblk[s] + GS * tg:
                                           sblk[s] + GS * tg + GS],
                                 rhs=tt[:, j * w:(j + 1) * w],
                                 start=(tg == 0), stop=(tg == in_g - 1))
                if tg == in_g - 1:
                    ow = SEGTN + FT if g == 0 else w
                    st = stage_p.tile([GS, ow], F32, tag="st", name=f"st{g}")
                    nc.vector.tensor_copy(st[:], outbs[g][:])
                    nc.gpsimd.dma_start(out=outs_d.ap()[g, :, :ow], in_=st[:])

    nc.compile()
    prog = (nc, pkey)
    _PROGRAM_CACHE[pkey] = prog
    return prog


# ---------------------------------------------------------------- kernel
def _prepare(seq_pads, background, W1, b1, W2, b2, W3, b3, W4, b4, seq_lens):
    t = np.asarray(seq_pads)[:, :, 0].astype(np.float32)
    lens = np.asarray(seq_lens).astype(np.int64)
    ft = _fits(
        np.asarray(W1, np.float64), np.asarray(b1, np.float64),
        np.asarray(W2, np.float64), np.asarray(b2, np.float64),
        np.asarray(W3, np.float64), np.asarray(b3, np.float64),
        np.asarray(W4, np.float64), np.asarray(b4, np.float64))

    xbb, (evn, evr), dims, nF = _pack(t, lens)
    TN, SEGTN, TR, SEGTR, FT = dims
    nc, _ = build_program(*dims)

    g16 = np.repeat(np.arange(8), SEGN)        # partition -> near/F rate
    g32 = np.repeat(np.arange(4), SEGR)        # partition -> far rate
    cfd = np.zeros((128, 6), np.float32)
    cfd[:, 0] = -ft["rn"][g16]
    cfd[:, 1] = np.log(np.maximum(np.abs(ft["cn"]), 1e-20))[g16]
    cfd[:, 2] = -ft["rF"][g16]
    cfd[:, 3] = np.log(np.maximum(np.abs(ft["cF"]), 1e-20))[g16]
    cfd[:, 5] = -ft["rf"][g32]
    cfd[:, 4] = np.log(np.maximum(np.abs(ft["cf"]), 1e-20))[g32]
    SELW = GS * GS * 2 + GS
    selv = np.zeros((128, SELW), np.float32)
    sgn_n = np.sign(ft["cn"])[g16]
    sgn_r = np.sign(ft["cf"])[g32]
    for v in range(GS):
        selv[:, GS * v + v] = sgn_n
        selv[:, GS * GS + GS * v + v] = sgn_r
    selv[:, 2 * GS * GS] = np.sign(ft["cF"])[g16]
    selv = selv.astype(NPBF16)

    in_maps = []
    for cix in range(NC):
        m = dict(selv=selv, cfd=cfd)
        m["xbb"] = np.ascontiguousarray(xbb[cix].astype(NPBF16))
        in_maps.append(m)
    meta = dict(evn=evn, evr=evr, dims=dims, nF=nF, CF=ft["CF"], lens=lens)
    return nc, in_maps, meta


def kernel(seq_pads, background, W1, b1, W2, b2, W3, b3, W4, b4, seq_lens):
    nc, in_maps, meta = _prepare(seq_pads, background, W1, b1, W2, b2, W3,
                                 b3, W4, b4, seq_lens)
    TN, SEGTN, TR, SEGTR, FT = meta["dims"]
    NGn = (TN + GS - 1) // GS
    NGr = (TR + GS - 1) // GS
    lens = meta["lens"]

    def decode(res):
        """-> near partials [NC*TN*SEGTN], far [NC*TR*SEGTR], f_exp."""
        pn, pr, fe = [], [], 0.0
        for cc in range(NC):
            o = res[cc]["out_s"]
            for g in range(NGn):
                in_g = min(GS, TN - GS * g)
                pn.append(o[g, :in_g, :SEGTN].reshape(-1))
            for g in range(NGr):
                in_g = min(GS, TR - GS * g)
                pr.append(o[NGn + g, :in_g, :SEGTR].reshape(-1))
            fe += o[0, :, SEGTN:SEGTN + FT].sum()
        return np.concatenate(pn), np.concatenate(pr), fe

    res = run_bass_kernel_spmd(nc, in_maps, core_ids=list(range(NC))).results
    pn, pr, f_exp = decode(res)
    if not (np.isfinite(pn).all() and np.isfinite(pr).all()
            and np.isfinite(f_exp)):
        res = run_bass_kernel_spmd(nc, in_maps,
                                   core_ids=list(range(NC))).results
        pn, pr, f_exp = decode(res)

    S = np.zeros(B * L, np.float64)
    for ev, p in ((meta["evn"].reshape(-1), pn), (meta["evr"].reshape(-1),
                                                  pr)):
        ok = ev >= 0
        np.add.at(S, ev[ok], p[ok].astype(np.float64))
    S = S.reshape(B, L)

    bg = float(np.asarray(background)[0])
    lam = bg + S
    mask = np.arange(L)[None, :] < lens[:, None]
    sum_log = np.log(np.where(mask, lam, 1.0)).sum()

    ints_total = f_exp + meta["nF"] * meta["CF"] + B * T_END * bg
    nll = -(sum_log - ints_total) / B
    return np.float32(nll)


# revision 20
# speedup vs baseline: 1.4499x; 1.1630x over previous
"""Trainium2 Bass kernel for nn_AutoIntTPPSameInfluence — exp-sum formulation.

dF(x) (the scalar derivative of the 1->64->64->64->1 tanh MLP) is fit on host
as sums of decaying exponentials, split by range:
  near (x < X0):   8 rates, 16 points per segment column (8*16 = 128)
  far (X0<=x<W):   4 rates, 32 points per segment column (4*32 = 128)
  x >= W:          dropped (dF tail integral < 1e-4 -> per-lambda error ~3e-4)
On device every pairwise influence point is ONE table-exp evaluation: points
are packed SEG-per-column and replicated over the partition groups of a
[128, N] tile, a single ACT instruction computes |c_i| exp(-s_i x + ln|c_i|)
for all rates via per-partition scale/bias, and one bf16 matmul with a +-1
selector contracts all 128 partitions — performing the SEG-point segment sum
AND the weighted rate sum at 1 col/cycle.  Masked/padded points use x = 3e4,
driving every exponential to exactly 0.

The integral term sum_k F(T_END - t_k) - F0 only needs a bulk sum, so
F(x) - F0 is fit the same way (const + 8 exps) and rides through the same
pipeline as the leading FT columns; the constant is applied on host via the
valid count.  Host: scatter segment sums to events, log/mask/reduce in f64.

Schedule notes: a dummy ACT on a framework const AP forces the exp
ACT_TABLE_LOAD to run during the input-DMA completion window (~2.5 us fixed
DMA latency); const DMAs go first on the sync queue and are consumed without
staging copies; F columns lead so their chain never sits in the tail; PSUM
accumulation groups let output DMAs start mid-run.
"""

import numpy as np
from contextlib import ExitStack

import ml_dtypes

import concourse.bass as bass
import concourse.bacc as bacc
import concourse.tile as tile
import concourse.mybir as mybir
from concourse.bass_utils import run_bass_kernel_spmd

B, L, H = 16, 320, 64
T_END = 100.0
NC = 8
SEGN = 16                   # near: points per column, 8 rates
SEGR = 32                   # far: points per column, 4 rates
X0 = 4.0                    # near/far boundary
WCUT = 12.0                 # truncation window
GS = 4                      # tiles per PSUM accumulation group
TS_MAX = 512                # PSUM bank width in f32
XPAD = np.float32(30000.0)  # pad x: exp(-s*XPAD) underflows to exactly 0
BF16 = mybir.dt.bfloat16
F32 = mybir.dt.float32
Exp = mybir.ActivationFunctionType.Exp
NPBF16 = ml_dtypes.bfloat16


# ---------------------------------------------------------------- host fits
_FIT_CACHE = {}


def _mlp_funcs(W1, b1, W2, b2, W3, b3, W4, b4):
    w1 = W1[:, 0].astype(np.float64)
    b1d, b2d, b3d = (b1.astype(np.float64), b2.astype(np.float64),
                     b3.astype(np.float64))
    W2d, W3d, W4d = (W2.astype(np.float64), W3.astype(np.float64),
                     W4.astype(np.float64))
    b4d = float(np.asarray(b4, np.float64)[0])

    def dF(x):
        x = np.ravel(x)
        h1 = np.outer(w1, x) + b1d[:, None]
        a1 = np.tanh(h1)
        d1 = (1 - a1 ** 2) * w1[:, None]
        h2 = W2d @ a1 + b2d[:, None]
        a2 = np.tanh(h2)
        d2 = (1 - a2 ** 2) * (W2d @ d1)
        h3 = W3d @ a2 + b3d[:, None]
        a3 = np.tanh(h3)
        d3 = (1 - a3 ** 2) * (W3d @ d2)
        return (W4d @ d3)[0]

    def F(x):
        x = np.ravel(x)
        h1 = np.tanh(np.outer(w1, x) + b1d[:, None])
        h2 = np.tanh(W2d @ h1 + b2d[:, None])
        h3 = np.tanh(W3d @ h2 + b3d[:, None])
        return (W4d @ h3)[0] + b4d

    return dF, F


def _ridge_fit(A, y, lam):
    cn = np.linalg.norm(A, axis=0)
    return np.linalg.solve(A.T @ A + lam * np.diag(cn ** 2), A.T @ y)


def _scan_fit(gx, gy, floor, nk, smins, smaxs, cmax):
    wts = 1.0 / (np.abs(gy) + floor)
    best = None
    for smin in smins:
        for smax in smaxs:
            r = np.geomspace(smin, smax, nk)
            A = np.exp(-np.outer(gx, r)) * wts[:, None]
            c = _ridge_fit(A, gy * wts, 1e-5)
            if np.abs(c).max() > cmax:
                continue
            werr = np.abs((np.exp(-np.outer(gx, r)) @ c - gy) * wts).max()
            if best is None or werr < best[0]:
                best = (werr, r, c)
    return best[1], best[2]


def _fits(W1, b1, W2, b2, W3, b3, W4, b4):
    key = b"".join(np.ascontiguousarray(a).tobytes()
                   for a in (W1, b1, W2, b2, W3, b3, W4, b4))
    if key in _FIT_CACHE:
        return _FIT_CACHE[key]
    dF, F = _mlp_funcs(W1, b1, W2, b2, W3, b3, W4, b4)
    F0 = float(F(np.zeros(1))[0])

    gx = np.unique(np.concatenate([np.geomspace(1e-4, X0 * 1.02, 4000),
                                   np.linspace(0, X0 * 1.02, 4000)]))
    rn, cn = _scan_fit(gx, dF(gx), 1e-4, 8,
                       (0.3, 0.5, 0.8, 1.2), (10., 14., 18., 24., 30.), 300.)
    gxf = np.linspace(X0 * 0.98, WCUT * 1.02, 6000)
    rf, cf = _scan_fit(gxf, dF(gxf), 2e-6, 4,
                       (0.1, 0.2, 0.3, 0.5), (1.0, 1.5, 2.5, 4.0), 1e3)

    gxF = np.linspace(0, 100, 20001)
    gyF = F(gxF) - F0
    rF = np.geomspace(0.04, 16.0, 8)
    AF = np.concatenate([np.ones((len(gxF), 1)),
                         np.exp(-np.outer(gxF, rF))], axis=1)
    cfF = _ridge_fit(AF, gyF, 1e-7)
    CF, cF = float(cfF[0]), cfF[1:]

    out = dict(rn=rn, cn=cn, rf=rf, cf=cf, rF=rF, cF=cF, CF=CF, F0=F0)
    _FIT_CACHE[key] = out
    return out


# ---------------------------------------------------------------- packing
def _seg_stream(t, lens, lo_f, hi_f, seg):
    """Segments of `seg` consecutive j's with t_k - t_j in [lo, hi) per event.
    Returns x [Tseg, seg] f32 (XPAD-padded), ev [Tseg] (b*L + k)."""
    xs, evs = [], []
    for b in range(B):
        n = int(lens[b])
        tb = t[b, :n].astype(np.float64)
        jhi = np.searchsorted(tb, tb - lo_f) if lo_f > 0 else np.arange(n)
        jlo = np.searchsorted(tb, tb - hi_f) if hi_f is not None else \
            np.zeros(n, np.int64)
        jhi = np.minimum(jhi, np.arange(n))
        cnt = jhi - jlo
        nsg = (cnt + seg - 1) // seg
        ev_idx = np.repeat(np.arange(n), nsg)
        starts = np.concatenate([[0], np.cumsum(nsg)[:-1]])
        within = (np.arange(int(nsg.sum())) - np.repeat(starts, nsg)) * seg
        j0 = jlo[ev_idx] + within
        jj = j0[:, None] + np.arange(seg)[None, :]
        valid = jj < jhi[ev_idx][:, None]
        jc = np.minimum(jj, n - 1)
        x = np.where(valid, tb[ev_idx][:, None] * 0 + (t[b, ev_idx][:, None]
                     - tb[jc]), XPAD).astype(np.float32)
        xs.append(x)
        evs.append(b * L + ev_idx)
    return np.concatenate(xs), np.concatenate(evs)


def _layout_stream(x, ev, seg, nrep):
    """Split stream across cores; -> xcore [NC, 128, cols_pc], ev [NC, cols],
    (NT, SEGT)."""
    Tseg = x.shape[0]
    Spc = (Tseg + NC - 1) // NC
    NT = (Spc + TS_MAX - 1) // TS_MAX
    SEGT = ((Spc + NT - 1) // NT + 7) // 8 * 8
    cap = NC * NT * SEGT
    xp = np.full((cap, seg), XPAD, np.float32)
    xp[:Tseg] = x
    evp = np.full(cap, -1, np.int64)
    evp[:Tseg] = ev
    xc = xp.reshape(NC, NT * SEGT, seg).transpose(0, 2, 1)  # [NC, seg, cols]
    xc = np.tile(xc, (1, nrep, 1))                          # [NC, 128, cols]
    return xc, evp.reshape(NC, NT * SEGT), NT, SEGT


def _pack(t, lens):
    xn, evn = _seg_stream(t, lens, 0.0, X0, SEGN)
    xr, evr = _seg_stream(t, lens, X0, WCUT, SEGR)
    xcn, evn, TN, SEGTN = _layout_stream(xn, evn, SEGN, 128 // SEGN)
    xcr, evr, TR, SEGTR = _layout_stream(xr, evr, SEGR, 128 // SEGR)

    # F points: one per valid event (all k < n)
    fx = []
    for b in range(B):
        n = int(lens[b])
        fx.append((T_END - t[b, :n]).astype(np.float32))
    fx = np.concatenate(fx)
    nF = len(fx)
    nFc = (nF + NC - 1) // NC
    FT = ((nFc + SEGN - 1) // SEGN + 1) // 2 * 2
    xf = np.full((NC * FT * SEGN,), XPAD, np.float32)
    xf[:nF] = fx
    xf = xf.reshape(NC, FT, SEGN).transpose(0, 2, 1)
    xf = np.tile(xf, (1, 128 // SEGN, 1))                   # [NC, 128, FT]

    assert SEGTN + FT <= TS_MAX
    xbb = np.concatenate([xf, xcn, xcr], axis=2)            # [NC, 128, XC]
    return xbb, (evn, evr), (TN, SEGTN, TR, SEGTR, FT), nF


# ---------------------------------------------------------------- program
_PROGRAM_CACHE = {}


def build_program(TN, SEGTN, TR, SEGTR, FT):
    pkey = (TN, SEGTN, TR, SEGTR, FT)
    if pkey in _PROGRAM_CACHE:
        return _PROGRAM_CACHE[pkey]
    XC = FT + TN * SEGTN + TR * SEGTR
    NGn = (TN + GS - 1) // GS
    NGr = (TR + GS - 1) // GS
    OUTW = NGr * SEGTR + NGn * SEGTN + FT    # single staged output row
    SELW = GS * GS * 2 + GS
    nc = bacc.Bacc("TRN2", target_bir_lowering=False, debug=False,
                   enable_asserts=False)

    xbb_d = nc.dram_tensor("xbb", [128, XC], BF16, kind="ExternalInput")
    selv_d = nc.dram_tensor("selv", [128, SELW], BF16, kind="ExternalInput")
    cf_d = nc.dram_tensor("cfd", [128, 6], F32, kind="ExternalInput")
    outs_d = nc.dram_tensor("out_s", [GS, OUTW], F32, kind="ExternalOutput")

    # tiles: ("R", i) far first, ("N", i) near last; chunks pair tiles
    # within a stream; the F columns get their own leading chunk
    tiles = [("R", i) for i in range(TR)] + [("N", i) for i in range(TN)]
    chunks = []
    i = 0
    while i < len(tiles):
        if i + 1 < len(tiles) and tiles[i][0] == tiles[i + 1][0]:
            chunks.append(tiles[i:i + 2])
            i += 2
        else:
            chunks.append(tiles[i:i + 1])
            i += 1

    def col0(tl):
        s, i = tl
        return FT + i * SEGTN if s == "N" else FT + TN * SEGTN + i * SEGTR

    def width(tl):
        return SEGTN if tl[0] == "N" else SEGTR

    with tile.TileContext(nc) as tc, ExitStack() as ctx, \
            nc.allow_low_precision(reason="bf16 exp terms; tol is 2e-2"):
        consts = ctx.enter_context(tc.tile_pool(name="consts", bufs=1))
        xb_p = ctx.enter_context(tc.tile_pool(name="xb", bufs=len(chunks)))
        term_p = ctx.enter_context(tc.tile_pool(name="term", bufs=3))
        outp_p = ctx.enter_context(tc.tile_pool(name="outp", bufs=4,
                                                space="PSUM"))
        stage_p = ctx.enter_context(tc.tile_pool(name="stage", bufs=2))

        # const DMAs ride the scalar queue ahead of the dummy ACT, so the
        # compiler-inserted ACT_TABLE_LOAD for Exp (bound to the dummy, which
        # has zero data deps) overlaps their ~2.5 us completion latency
        cfc = consts.tile([128, 6], F32, tag="cfc")
        nc.scalar.dma_start(out=cfc[:], in_=cf_d.ap(), single_packet=True)
        selc = consts.tile([128, SELW], BF16, tag="selc")
        nc.scalar.dma_start(out=selc[:], in_=selv_d.ap())
        zeros_ap = nc.const_aps.aps[(mybir.dt.float32, 0.0)]
        dummy = consts.tile([128, 1], F32, tag="dummy")
        nc.scalar.activation(dummy[:], zeros_ap, Exp)

        # F columns: own small leading DMA + the whole seg region behind it
        xbf = xb_p.tile([128, FT], BF16, tag="xbf")
        nc.sync.dma_start(out=xbf[:], in_=xbb_d.ap()[:, 0:FT])
        xbts = []
        for ci, ch in enumerate(chunks):
            a = col0(ch[0])
            z = col0(ch[-1]) + width(ch[-1])
            xbt = xb_p.tile([128, z - a], BF16, tag="xb", name=f"xb{ci}")
            nc.sync.dma_start(out=xbt[:], in_=xbb_d.ap()[:, a:z])
            xbts.append((z - a, xbt))

        # F chain first: its ACT/MM/copy/DMA never sit in the tail
        termf = term_p.tile([128, FT], BF16, tag="termf")
        nc.scalar.activation(termf[:], xbf[:], Exp,
                             bias=cfc[:, 3:4], scale=cfc[:, 2:3])

        cpair = {"N": (cfc[:, 0:1], cfc[:, 1:2]),
                 "R": (cfc[:, 5:6], cfc[:, 4:5])}
        sblk = {"N": 0, "R": GS * GS}
        gF = NGr                                   # F rides near group 0

        def gcol(g):                               # group -> stage col offset
            if g < NGr:
                return g * SEGTR
            return NGr * SEGTR + (SEGTN + FT if g > gF else 0) \
                + (g - NGr - 1 if g > gF else 0) * SEGTN

        def gwidth(g):
            if g < NGr:
                return SEGTR
            return SEGTN + FT if g == gF else SEGTN

        stage = stage_p.tile([GS, OUTW], F32, tag="stage")
        outbs = {}
        for ci, ch in enumerate(chunks):
            cw, xbt = xbts[ci]
            tt = term_p.tile([128, cw], BF16, tag="terms",
                             name=f"terms{ci}")
            sc, bi = cpair[ch[0][0]]
            nc.scalar.activation(tt[:], xbt[:], Exp, bias=bi, scale=sc)
            for j, tl in enumerate(ch):
                s, i = tl
                w = width(tl)
                g = i // GS + (NGr if s == "N" else 0)
                tg = i % GS
                in_g = min(GS, (TN if s == "N" else TR) - GS * (i // GS))
                if tg == 0:
                    outbs[g] = outp_p.tile([GS, gwidth(g)], F32, tag="outb",
                                           name=f"outb{g}")
                    if g == gF:
                        nc.tensor.matmul(out=outbs[g][:, SEGTN:],
                                         lhsT=selc[:, 2 * GS * GS:],
                                         rhs=termf[:], start=True, stop=True)
                nc.tensor.matmul(out=outbs[g][:, :w],
                                 lhsT=selc[:, sblk[s] + GS * tg:
                                           sblk[s] + GS * tg + GS],
                                 rhs=tt[:, j * w:(j + 1) * w],
                                 start=(tg == 0), stop=(tg == in_g - 1))
                if tg == in_g - 1:
                    nc.vector.tensor_copy(
                        stage[:, gcol(g):gcol(g) + gwidth(g)], outbs[g][:])
        nc.sync.dma_start(out=outs_d.ap(), in_=stage[:])

    nc.compile()
    prog = (nc, pkey)
    _PROGRAM_CACHE[pkey] = prog
    return prog


# ---------------------------------------------------------------- kernel
def _prepare(seq_pads, background, W1, b1, W2, b2, W3, b3, W4, b4, seq_lens):
    t = np.asarray(seq_pads)[:, :, 0].astype(np.float32)
    lens = np.asarray(seq_lens).astype(np.int64)
    ft = _fits(
        np.asarray(W1, np.float64), np.asarray(b1, np.float64),
        np.asarray(W2, np.float64), np.asarray(b2, np.float64),
        np.asarray(W3, np.float64), np.asarray(b3, np.float64),
        np.asarray(W4, np.float64), np.asarray(b4, np.float64))

    xbb, (evn, evr), dims, nF = _pack(t, lens)
    TN, SEGTN, TR, SEGTR, FT = dims
    nc, _ = build_program(*dims)

    g16 = np.repeat(np.arange(8), SEGN)        # partition -> near/F rate
    g32 = np.repeat(np.arange(4), SEGR)        # partition -> far rate
    cfd = np.zeros((128, 6), np.float32)
    cfd[:, 0] = -ft["rn"][g16]
    cfd[:, 1] = np.log(np.maximum(np.abs(ft["cn"]), 1e-20))[g16]
    cfd[:, 2] = -ft["rF"][g16]
    cfd[:, 3] = np.log(np.maximum(np.abs(ft["cF"]), 1e-20))[g16]
    cfd[:, 5] = -ft["rf"][g32]
    cfd[:, 4] = np.log(np.maximum(np.abs(ft["cf"]), 1e-20))[g32]
    SELW = GS * GS * 2 + GS
    selv = np.zeros((128, SELW), np.float32)
    sgn_n = np.sign(ft["cn"])[g16]
    sgn_r = np.sign(ft["cf"])[g32]
    for v in range(GS):
        selv[:, GS * v + v] = sgn_n
        selv[:, GS * GS + GS * v + v] = sgn_r
    selv[:, 2 * GS * GS] = np.sign(ft["cF"])[g16]
    selv = selv.astype(NPBF16)

    in_maps = []
    for cix in range(NC):
        m = dict(selv=selv, cfd=cfd)
        m["xbb"] = np.ascontiguousarray(xbb[cix].astype(NPBF16))
        in_maps.append(m)
    meta = dict(evn=evn, evr=evr, dims=dims, nF=nF, CF=ft["CF"], lens=lens)
    return nc, in_maps, meta


def kernel(seq_pads, background, W1, b1, W2, b2, W3, b3, W4, b4, seq_lens):
    nc, in_maps, meta = _prepare(seq_pads, background, W1, b1, W2, b2, W3,
                                 b3, W4, b4, seq_lens)
    TN, SEGTN, TR, SEGTR, FT = meta["dims"]
    NGn = (TN + GS - 1) // GS
    NGr = (TR + GS - 1) // GS
    lens = meta["lens"]

    def decode(res):
        """-> near partials [NC*TN*SEGTN], far [NC*TR*SEGTR], f_exp."""
        pn, pr, fe = [], [], 0.0
        for cc in range(NC):
            o = res[cc]["out_s"]                   # [GS, OUTW]
            for g in range(NGr):
                in_g = min(GS, TR - GS * g)
                pr.append(o[:in_g, g * SEGTR:(g + 1) * SEGTR].reshape(-1))
            base = NGr * SEGTR
            for g in range(NGn):
                c0 = base + (SEGTN + FT) * min(g, 1) + max(g - 1, 0) * SEGTN
                in_g = min(GS, TN - GS * g)
                pn.append(o[:in_g, c0:c0 + SEGTN].reshape(-1))
            fe += o[:, base + SEGTN:base + SEGTN + FT].sum()
        return np.concatenate(pn), np.concatenate(pr), fe

    res = run_bass_kernel_spmd(nc, in_maps, core_ids=list(range(NC))).results
    pn, pr, f_exp = decode(res)
    if not (np.isfinite(pn).all() and np.isfinite(pr).all()
            and np.isfinite(f_exp)):
        res = run_bass_kernel_spmd(nc, in_maps,
                                   core_ids=list(range(NC))).results
        pn, pr, f_exp = decode(res)

    S = np.zeros(B * L, np.float64)
    for ev, p in ((meta["evn"].reshape(-1), pn), (meta["evr"].reshape(-1),
                                                  pr)):
        ok = ev >= 0
        np.add.at(S, ev[ok], p[ok].astype(np.float64))
    S = S.reshape(B, L)

    bg = float(np.asarray(background)[0])
    lam = bg + S
    mask = np.arange(L)[None, :] < lens[:, None]
    sum_log = np.log(np.where(mask, lam, 1.0)).sum()

    ints_total = f_exp + meta["nF"] * meta["CF"] + B * T_END * bg
    nll = -(sum_log - ints_total) / B
    return np.float32(nll)


# revision 26
# speedup vs baseline: 1.5014x; 1.0355x over previous
"""Trainium2 Bass kernel for nn_AutoIntTPPSameInfluence — exp-sum formulation.

dF(x) (the scalar derivative of the 1->64->64->64->1 tanh MLP) is fit on host
as sums of decaying exponentials, split by range:
  near (x < X0):   8 rates, 16 points per segment column (8*16 = 128)
  far (X0<=x<W):   4 rates, 32 points per segment column (4*32 = 128)
  x >= W:          dropped (dF tail integral < 1e-4 -> per-lambda error ~3e-4)
On device every pairwise influence point is ONE table-exp evaluation: points
are packed SEG-per-column and replicated over the partition groups of a
[128, N] tile, a single ACT instruction computes |c_i| exp(-s_i x + ln|c_i|)
for all rates via per-partition scale/bias, and one bf16 matmul with a +-1
selector contracts all 128 partitions — performing the SEG-point segment sum
AND the weighted rate sum at 1 col/cycle.  Masked/padded points use x = 3e4,
driving every exponential to exactly 0.

The integral term sum_k F(T_END - t_k) - F0 only needs a bulk sum, so
F(x) - F0 is fit the same way (const + 8 exps) and rides through the same
pipeline as the leading FT columns; the constant is applied on host via the
valid count.  Host: scatter segment sums to events, log/mask/reduce in f64.

Schedule notes: a dummy ACT on a framework const AP forces the exp
ACT_TABLE_LOAD to run during the input-DMA completion window (~2.5 us fixed
DMA latency); const DMAs go first on the sync queue and are consumed without
staging copies; F columns lead so their chain never sits in the tail; PSUM
accumulation groups let output DMAs start mid-run.
"""

import numpy as np
from contextlib import ExitStack

import ml_dtypes

import concourse.bass as bass
import concourse.bacc as bacc
import concourse.tile as tile
import concourse.mybir as mybir
from concourse.bass_utils import run_bass_kernel_spmd

B, L, H = 16, 320, 64
T_END = 100.0
NC = 8
SEGN = 16                   # near: points per column, 8 rates
SEGR = 32                   # far: points per column, 4 rates
X0 = 4.0                    # near/far boundary
WCUT = 12.0                 # truncation window
GS = 4                      # tiles per PSUM accumulation group
TS_MAX = 512                # PSUM bank width in f32
XPAD = np.float32(30000.0)  # pad x: exp(-s*XPAD) underflows to exactly 0
BF16 = mybir.dt.bfloat16
F32 = mybir.dt.float32
Exp = mybir.ActivationFunctionType.Exp
NPBF16 = ml_dtypes.bfloat16


# ---------------------------------------------------------------- host fits
_FIT_CACHE = {}


def _mlp_funcs(W1, b1, W2, b2, W3, b3, W4, b4):
    w1 = W1[:, 0].astype(np.float64)
    b1d, b2d, b3d = (b1.astype(np.float64), b2.astype(np.float64),
                     b3.astype(np.float64))
    W2d, W3d, W4d = (W2.astype(np.float64), W3.astype(np.float64),
                     W4.astype(np.float64))
    b4d = float(np.asarray(b4, np.float64)[0])

    def dF(x):
        x = np.ravel(x)
        h1 = np.outer(w1, x) + b1d[:, None]
        a1 = np.tanh(h1)
        d1 = (1 - a1 ** 2) * w1[:, None]
        h2 = W2d @ a1 + b2d[:, None]
        a2 = np.tanh(h2)
        d2 = (1 - a2 ** 2) * (W2d @ d1)
        h3 = W3d @ a2 + b3d[:, None]
        a3 = np.tanh(h3)
        d3 = (1 - a3 ** 2) * (W3d @ d2)
        return (W4d @ d3)[0]

    def F(x):
        x = np.ravel(x)
        h1 = np.tanh(np.outer(w1, x) + b1d[:, None])
        h2 = np.tanh(W2d @ h1 + b2d[:, None])
        h3 = np.tanh(W3d @ h2 + b3d[:, None])
        return (W4d @ h3)[0] + b4d

    return dF, F


def _ridge_fit(A, y, lam):
    cn = np.linalg.norm(A, axis=0)
    return np.linalg.solve(A.T @ A + lam * np.diag(cn ** 2), A.T @ y)


def _scan_fit(gx, gy, floor, nk, smins, smaxs, cmax):
    wts = 1.0 / (np.abs(gy) + floor)
    best = None
    for smin in smins:
        for smax in smaxs:
            r = np.geomspace(smin, smax, nk)
            A = np.exp(-np.outer(gx, r)) * wts[:, None]
            c = _ridge_fit(A, gy * wts, 1e-5)
            if np.abs(c).max() > cmax:
                continue
            werr = np.abs((np.exp(-np.outer(gx, r)) @ c - gy) * wts).max()
            if best is None or werr < best[0]:
                best = (werr, r, c)
    return best[1], best[2]


def _fits(W1, b1, W2, b2, W3, b3, W4, b4):
    key = b"".join(np.ascontiguousarray(a).tobytes()
                   for a in (W1, b1, W2, b2, W3, b3, W4, b4))
    if key in _FIT_CACHE:
        return _FIT_CACHE[key]
    dF, F = _mlp_funcs(W1, b1, W2, b2, W3, b3, W4, b4)
    F0 = float(F(np.zeros(1))[0])

    gx = np.unique(np.concatenate([np.geomspace(1e-4, X0 * 1.02, 4000),
                                   np.linspace(0, X0 * 1.02, 4000)]))
    rn, cn = _scan_fit(gx, dF(gx), 1e-4, 8,
                       (0.3, 0.5, 0.8, 1.2), (10., 14., 18., 24., 30.), 300.)
    gxf = np.linspace(X0 * 0.98, WCUT * 1.02, 6000)
    rf, cf = _scan_fit(gxf, dF(gxf), 2e-6, 4,
                       (0.1, 0.2, 0.3, 0.5), (1.0, 1.5, 2.5, 4.0), 1e3)

    gxF = np.linspace(0, 100, 20001)
    gyF = F(gxF) - F0
    rF = np.geomspace(0.04, 16.0, 8)
    AF = np.concatenate([np.ones((len(gxF), 1)),
                         np.exp(-np.outer(gxF, rF))], axis=1)
    cfF = _ridge_fit(AF, gyF, 1e-7)
    CF, cF = float(cfF[0]), cfF[1:]

    out = dict(rn=rn, cn=cn, rf=rf, cf=cf, rF=rF, cF=cF, CF=CF, F0=F0)
    _FIT_CACHE[key] = out
    return out


# ---------------------------------------------------------------- packing
def _seg_stream(t, lens, lo_f, hi_f, seg):
    """Segments of `seg` consecutive j's with t_k - t_j in [lo, hi) per event.
    Returns x [Tseg, seg] f32 (XPAD-padded), ev [Tseg] (b*L + k)."""
    xs, evs = [], []
    for b in range(B):
        n = int(lens[b])
        tb = t[b, :n].astype(np.float64)
        jhi = np.searchsorted(tb, tb - lo_f) if lo_f > 0 else np.arange(n)
        jlo = np.searchsorted(tb, tb - hi_f) if hi_f is not None else \
            np.zeros(n, np.int64)
        jhi = np.minimum(jhi, np.arange(n))
        cnt = jhi - jlo
        nsg = (cnt + seg - 1) // seg
        ev_idx = np.repeat(np.arange(n), nsg)
        starts = np.concatenate([[0], np.cumsum(nsg)[:-1]])
        within = (np.arange(int(nsg.sum())) - np.repeat(starts, nsg)) * seg
        j0 = jlo[ev_idx] + within
        jj = j0[:, None] + np.arange(seg)[None, :]
        valid = jj < jhi[ev_idx][:, None]
        jc = np.minimum(jj, n - 1)
        x = np.where(valid, tb[ev_idx][:, None] * 0 + (t[b, ev_idx][:, None]
                     - tb[jc]), XPAD).astype(np.float32)
        xs.append(x)
        evs.append(b * L + ev_idx)
    return np.concatenate(xs), np.concatenate(evs)


def _layout_stream(x, ev, seg, nrep):
    """Split stream across cores; -> xcore [NC, 128, cols_pc], ev [NC, cols],
    (NT, SEGT)."""
    Tseg = x.shape[0]
    Spc = (Tseg + NC - 1) // NC
    NT = (Spc + TS_MAX - 1) // TS_MAX
    SEGT = ((Spc + NT - 1) // NT + 7) // 8 * 8
    cap = NC * NT * SEGT
    xp = np.full((cap, seg), XPAD, np.float32)
    xp[:Tseg] = x
    evp = np.full(cap, -1, np.int64)
    evp[:Tseg] = ev
    xc = xp.reshape(NC, NT * SEGT, seg).transpose(0, 2, 1)  # [NC, seg, cols]
    xc = np.tile(xc, (1, nrep, 1))                          # [NC, 128, cols]
    return xc, evp.reshape(NC, NT * SEGT), NT, SEGT


def _pack(t, lens):
    xn, evn = _seg_stream(t, lens, 0.0, X0, SEGN)
    xr, evr = _seg_stream(t, lens, X0, WCUT, SEGR)
    xcn, evn, TN, SEGTN = _layout_stream(xn, evn, SEGN, 128 // SEGN)
    xcr, evr, TR, SEGTR = _layout_stream(xr, evr, SEGR, 128 // SEGR)

    # F points: one per valid event (all k < n)
    fx = []
    for b in range(B):
        n = int(lens[b])
        fx.append((T_END - t[b, :n]).astype(np.float32))
    fx = np.concatenate(fx)
    nF = len(fx)
    nFc = (nF + NC - 1) // NC
    FT = ((nFc + SEGN - 1) // SEGN + 1) // 2 * 2
    xf = np.full((NC * FT * SEGN,), XPAD, np.float32)
    xf[:nF] = fx
    xf = xf.reshape(NC, FT, SEGN).transpose(0, 2, 1)
    xf = np.tile(xf, (1, 128 // SEGN, 1))                   # [NC, 128, FT]

    assert SEGTN + FT <= TS_MAX
    xbb = np.concatenate([xf, xcr, xcn], axis=2)            # [NC, 128, XC]
    return xbb, (evn, evr), (TN, SEGTN, TR, SEGTR, FT), nF


# ---------------------------------------------------------------- program
_PROGRAM_CACHE = {}


def build_program(TN, SEGTN, TR, SEGTR, FT):
    pkey = (TN, SEGTN, TR, SEGTR, FT)
    if pkey in _PROGRAM_CACHE:
        return _PROGRAM_CACHE[pkey]
    XC = FT + TN * SEGTN + TR * SEGTR
    NGn = (TN + GS - 1) // GS
    NGr = (TR + GS - 1) // GS
    OUTW = NGr * SEGTR + NGn * SEGTN + FT    # single staged output row
    SELW = GS * GS * 2 + GS
    nc = bacc.Bacc("TRN2", target_bir_lowering=False, debug=False,
                   enable_asserts=False)

    xbb_d = nc.dram_tensor("xbb", [128, XC], BF16, kind="ExternalInput")
    selv_d = nc.dram_tensor("selv", [128, SELW], BF16, kind="ExternalInput")
    cf_d = nc.dram_tensor("cfd", [128, 6], F32, kind="ExternalInput")
    outs_d = nc.dram_tensor("out_s", [GS, OUTW], F32, kind="ExternalOutput")

    # tiles: ("R", i) far first, ("N", i) near last; chunks pair tiles
    # within a stream; the F columns get their own leading chunk
    tiles = [("R", i) for i in range(TR)] + [("N", i) for i in range(TN)]
    chunks = []
    i = 0
    while i < len(tiles):
        if i + 1 < len(tiles) and tiles[i][0] == tiles[i + 1][0]:
            chunks.append(tiles[i:i + 2])
            i += 2
        else:
            chunks.append(tiles[i:i + 1])
            i += 1

    def col0(tl):
        s, i = tl
        return FT + TR * SEGTR + i * SEGTN if s == "N" else FT + i * SEGTR

    def width(tl):
        return SEGTN if tl[0] == "N" else SEGTR

    with tile.TileContext(nc) as tc, ExitStack() as ctx, \
            nc.allow_low_precision(reason="bf16 exp terms; tol is 2e-2"):
        consts = ctx.enter_context(tc.tile_pool(name="consts", bufs=1))
        xb_p = ctx.enter_context(tc.tile_pool(name="xb", bufs=len(chunks)))
        term_p = ctx.enter_context(tc.tile_pool(name="term", bufs=3))
        outp_p = ctx.enter_context(tc.tile_pool(name="outp", bufs=4,
                                                space="PSUM"))
        stage_p = ctx.enter_context(tc.tile_pool(name="stage", bufs=2))

        # const DMAs ride the scalar queue ahead of the dummy ACT, so the
        # compiler-inserted ACT_TABLE_LOAD for Exp (bound to the dummy, which
        # has zero data deps) overlaps their ~2.5 us completion latency
        cfc = consts.tile([128, 6], F32, tag="cfc")
        nc.scalar.dma_start(out=cfc[:], in_=cf_d.ap(), single_packet=True)
        selc = consts.tile([128, SELW], BF16, tag="selc")
        nc.scalar.dma_start(out=selc[:], in_=selv_d.ap())
        zeros_ap = nc.const_aps.aps[(mybir.dt.float32, 0.0)]
        dummy = consts.tile([128, 1], F32, tag="dummy")
        nc.scalar.activation(dummy[:], zeros_ap, Exp)

        # chunk 0 carries the FT leading F columns along with the first far
        # tiles so every ACT's data rides the earliest DMA completions
        xbts = []
        for ci, ch in enumerate(chunks):
            a = 0 if ci == 0 else col0(ch[0])
            z = col0(ch[-1]) + width(ch[-1])
            xbt = xb_p.tile([128, z - a], BF16, tag="xb", name=f"xb{ci}")
            nc.sync.dma_start(out=xbt[:], in_=xbb_d.ap()[:, a:z])
            xbts.append((z - a - (FT if ci == 0 else 0), xbt))

        # F chain first: its ACT/MM/copy/DMA never sit in the tail
        termf = term_p.tile([128, FT], BF16, tag="termf")
        nc.scalar.activation(termf[:], xbts[0][1][:, :FT], Exp,
                             bias=cfc[:, 3:4], scale=cfc[:, 2:3])

        cpair = {"N": (cfc[:, 0:1], cfc[:, 1:2]),
                 "R": (cfc[:, 5:6], cfc[:, 4:5])}
        sblk = {"N": 0, "R": GS * GS}
        gF = NGr                                   # F rides near group 0

        def gcol(g):                               # group -> stage col offset
            if g < NGr:
                return g * SEGTR
            return NGr * SEGTR + (SEGTN + FT if g > gF else 0) \
                + (g - NGr - 1 if g > gF else 0) * SEGTN

        def gwidth(g):
            if g < NGr:
                return SEGTR
            return SEGTN + FT if g == gF else SEGTN

        stage = stage_p.tile([GS, OUTW], F32, tag="stage")
        outbs = {}
        for ci, ch in enumerate(chunks):
            cw, xbt = xbts[ci]
            a = FT if ci == 0 else 0
            tt = term_p.tile([128, cw], BF16, tag="terms",
                             name=f"terms{ci}")
            sc, bi = cpair[ch[0][0]]
            nc.scalar.activation(tt[:], xbt[:, a:], Exp, bias=bi, scale=sc)
            for j, tl in enumerate(ch):
                s, i = tl
                w = width(tl)
                g = i // GS + (NGr if s == "N" else 0)
                tg = i % GS
                in_g = min(GS, (TN if s == "N" else TR) - GS * (i // GS))
                if tg == 0:
                    outbs[g] = outp_p.tile([GS, gwidth(g)], F32, tag="outb",
                                           name=f"outb{g}")
                    if g == gF:
                        nc.tensor.matmul(out=outbs[g][:, SEGTN:],
                                         lhsT=selc[:, 2 * GS * GS:],
                                         rhs=termf[:], start=True, stop=True)
                nc.tensor.matmul(out=outbs[g][:, :w],
                                 lhsT=selc[:, sblk[s] + GS * tg:
                                           sblk[s] + GS * tg + GS],
                                 rhs=tt[:, j * w:(j + 1) * w],
                                 start=(tg == 0), stop=(tg == in_g - 1))
                if tg == in_g - 1:
                    nc.vector.tensor_copy(
                        stage[:, gcol(g):gcol(g) + gwidth(g)], outbs[g][:])
        nc.sync.dma_start(out=outs_d.ap(), in_=stage[:])

    nc.compile()
    prog = (nc, pkey)
    _PROGRAM_CACHE[pkey] = prog
    return prog


# ---------------------------------------------------------------- kernel
def _prepare(seq_pads, background, W1, b1, W2, b2, W3, b3, W4, b4, seq_lens):
    t = np.asarray(seq_pads)[:, :, 0].astype(np.float32)
    lens = np.asarray(seq_lens).astype(np.int64)
    ft = _fits(
        np.asarray(W1, np.float64), np.asarray(b1, np.float64),
        np.asarray(W2, np.float64), np.asarray(b2, np.float64),
        np.asarray(W3, np.float64), np.asarray(b3, np.float64),
        np.asarray(W4, np.float64), np.asarray(b4, np.float64))

    xbb, (evn, evr), dims, nF = _pack(t, lens)
    TN, SEGTN, TR, SEGTR, FT = dims
    nc, _ = build_program(*dims)

    g16 = np.repeat(np.arange(8), SEGN)        # partition -> near/F rate
    g32 = np.repeat(np.arange(4), SEGR)        # partition -> far rate
    cfd = np.zeros((128, 6), np.float32)
    cfd[:, 0] = -ft["rn"][g16]
    cfd[:, 1] = np.log(np.maximum(np.abs(ft["cn"]), 1e-20))[g16]
    cfd[:, 2] = -ft["rF"][g16]
    cfd[:, 3] = np.log(np.maximum(np.abs(ft["cF"]), 1e-20))[g16]
    cfd[:, 5] = -ft["rf"][g32]
    cfd[:, 4] = np.log(np.maximum(np.abs(ft["cf"]), 1e-20))[g32]
    SELW = GS * GS * 2 + GS
    selv = np.zeros((128, SELW), np.float32)
    sgn_n = np.sign(ft["cn"])[g16]
    sgn_r = np.sign(ft["cf"])[g32]
    for v in range(GS):
        selv[:, GS * v + v] = sgn_n
        selv[:, GS * GS + GS * v + v] = sgn_r
    selv[:, 2 * GS * GS] = np.sign(ft["cF"])[g16]
    selv = selv.astype(NPBF16)

    in_maps = []
    for cix in range(NC):
        m = dict(selv=selv, cfd=cfd)
        m["xbb"] = np.ascontiguousarray(xbb[cix].astype(NPBF16))
        in_maps.append(m)
    meta = dict(evn=evn, evr=evr, dims=dims, nF=nF, CF=ft["CF"], lens=lens)
    return nc, in_maps, meta


def kernel(seq_pads, background, W1, b1, W2, b2, W3, b3, W4, b4, seq_lens):
    nc, in_maps, meta = _prepare(seq_pads, background, W1, b1, W2, b2, W3,
                                 b3, W4, b4, seq_lens)
    TN, SEGTN, TR, SEGTR, FT = meta["dims"]
    NGn = (TN + GS - 1) // GS
    NGr = (TR + GS - 1) // GS
    lens = meta["lens"]

    def decode(res):
        """-> near partials [NC*TN*SEGTN], far [NC*TR*SEGTR], f_exp."""
        pn, pr, fe = [], [], 0.0
        for cc in range(NC):
            o = res[cc]["out_s"]                   # [GS, OUTW]
            for g in range(NGr):
                in_g = min(GS, TR - GS * g)
                pr.append(o[:in_g, g * SEGTR:(g + 1) * SEGTR].reshape(-1))
            base = NGr * SEGTR
            for g in range(NGn):
                c0 = base + (SEGTN + FT) * min(g, 1) + max(g - 1, 0) * SEGTN
                in_g = min(GS, TN - GS * g)
                pn.append(o[:in_g, c0:c0 + SEGTN].reshape(-1))
            fe += o[:, base + SEGTN:base + SEGTN + FT].sum()
        return np.concatenate(pn), np.concatenate(pr), fe

    res = run_bass_kernel_spmd(nc, in_maps, core_ids=list(range(NC))).results
    pn, pr, f_exp = decode(res)
    if not (np.isfinite(pn).all() and np.isfinite(pr).all()
            and np.isfinite(f_exp)):
        res = run_bass_kernel_spmd(nc, in_maps,
                                   core_ids=list(range(NC))).results
        pn, pr, f_exp = decode(res)

    S = np.zeros(B * L, np.float64)
    for ev, p in ((meta["evn"].reshape(-1), pn), (meta["evr"].reshape(-1),
                                                  pr)):
        ok = ev >= 0
        np.add.at(S, ev[ok], p[ok].astype(np.float64))
    S = S.reshape(B, L)

    bg = float(np.asarray(background)[0])
    lam = bg + S
    mask = np.arange(L)[None, :] < lens[:, None]
    sum_log = np.log(np.where(mask, lam, 1.0)).sum()

    ints_total = f_exp + meta["nF"] * meta["CF"] + B * T_END * bg
    nll = -(sum_log - ints_total) / B
    return np.float32(nll)
